# revision 45
# baseline (speedup 1.0000x reference)
"""Trainium2 Bass kernel for the CPC loss problem (nn_CPC_85117661872355).

Strategy (data-parallel over batch B across 8 cores):
  - Each core handles 8 of the 64 batch elements: 1120 prediction rows.
  - pred = ctx @ Wk[s]^T + b on the PE as a single-pass bf16 matmul
    (fp32 PSUM accumulate).  Host-side numpy check against the fixed
    inputs shows 1-pass bf16 + fp16 targets reproduces the reference
    argmax count exactly and has loss rel-err ~3e-6.
  - All 17 logits per row (1 positive + 16 negatives) are dot products
    pred_row . enc_flat[idx].  Target vectors are fetched with SWDGE
    dma_gather from an fp16 copy of the encoding table and the dots are
    split across two engines: 4 slots/supergroup on DVE via the fused
    scalar_tensor_tensor, 13 slots as DVE tensor_tensor (fp16 2x mode)
    followed by an ACT Copy+accumulate readout.  Phase-2 work for each
    128-row supergroup is emitted as soon as phase 1 has produced its
    pred rows, so gather/DVE/ACT/PE all overlap.
  - Softmax-CE runs batched at the end (single Exp table load, one Ln)
    and the argmax==0 check on DVE; per-core (loss_sum, correct_sum)
    are reduced over partitions with a K=128 ones-matmul and DMA'd out
    as [1,2].  Host sums the 8 partial pairs and divides by n_preds.
"""

import contextlib
import functools

import ml_dtypes
import numpy as np

import concourse.bass as bass
import concourse.mybir as mybir
import concourse.tile as tile
from concourse import bacc
from concourse.bass_utils import run_bass_kernel_spmd

F32 = mybir.dt.float32
BF16 = mybir.dt.bfloat16
FP16 = mybir.dt.float16

B, G, D = 64, 7, 1280
S, NEG = 5, 16
NCORES = 8
BSH = B // NCORES  # 8
NS = [BSH * (6 - s) * G for s in range(S)]  # [336, 280, 224, 168, 112]
SOFF = [0]
for n in NS:
    SOFF.append(SOFF[-1] + n)
NR = SOFF[-1]  # 1120 rows per core
NSG = 9  # supergroups of 128 rows
NDOT = 17  # 1 positive + 16 negatives
T_S = [3, 3, 2, 2, 1]  # row-tiles per s
GCHUNKS = [(0, 6), (6, 6), (12, 5)]  # (goff, width)
N_STT = 6  # slots computed via DVE STT; the rest go TT(DVE) + accum(ACT)
PE_SG0 = 5  # supergroups >= this go through the PE all-pairs path
N_PESG = NSG - PE_SG0  # 4
GR = 64  # fp32 granule (256B) for the dots-result gather
BLK = 3136 // GR  # 49 granules per dots row
TABC = 448  # all-pairs tab-chunk width (3136 = 7*448)
IDX_PER_SG = NDOT * 128  # 2176
IDX_TOT = NSG * IDX_PER_SG  # 19584
N_PREDS = B * G * 20  # 8960

# Results of the last device run (for test harness introspection)
LAST_RUN = {}


@functools.lru_cache(maxsize=1)
def build_nc() -> bass.Bass:
    nc = bacc.Bacc(
        "TRN2",
        target_bir_lowering=False,
        debug=False,
        num_devices=NCORES,
        num_swdge_queues=4,
    )
    ctxTh = nc.declare_dram_parameter("ctxTh", [D, NR], BF16, isOutput=False)
    wkTh = nc.declare_dram_parameter("wkTh", [S, D, D], BF16, isOutput=False)
    wkbH = nc.declare_dram_parameter("wkbH", [1, S, D], BF16, isOutput=False)
    ench = nc.declare_dram_parameter("ench", [B * G * G, D], FP16, isOutput=False)
    encT = nc.declare_dram_parameter("encT", [D, B * G * G], FP16, isOutput=False)
    idx2 = nc.declare_dram_parameter(
        "idx2", [128, N_PESG * IDX_PER_SG // 16], mybir.dt.int16, isOutput=False
    )
    off2 = nc.declare_dram_parameter("off2", [128, N_PESG * NDOT], F32, isOutput=False)
    iota64 = nc.declare_dram_parameter("iota64", [128, GR], F32, isOutput=False)
    idx = nc.declare_dram_parameter(
        "idx", [128, IDX_TOT // 16], mybir.dt.int16, isOutput=False
    )
    out = nc.declare_dram_parameter("out", [128, 2], F32, isOutput=True)

    Alu = mybir.AluOpType
    Act = mybir.ActivationFunctionType
    Ax = mybir.AxisListType

    with tile.TileContext(nc) as tc, contextlib.ExitStack() as es:
        def pool(**kw):
            return es.enter_context(tc.tile_pool(**kw))
        constp = pool(name="const", bufs=1)
        wkp = pool(name="wk", bufs=2)
        predp = pool(name="pred", bufs=NSG)
        stagep = pool(name="stage", bufs=1)
        gathp = pool(name="gath", bufs=3)
        scrp = pool(name="scr", bufs=1)
        scr2p = pool(name="scr2", bufs=3)
        dsttp = pool(name="dstt", bufs=NSG)
        dactp = pool(name="dact", bufs=NSG)
        smallp = pool(name="small", bufs=4)
        accp = pool(name="acc", bufs=1)
        predTp = pool(name="predT", bufs=N_PESG)
        enccp = pool(name="encc", bufs=2)
        dsbp = pool(name="dsb", bufs=2)
        gt2p = pool(name="gt2", bufs=2)
        dallp = pool(name="dall", bufs=N_PESG)
        psump = pool(name="psum", bufs=2, space="PSUM")
        psum2p = pool(name="psum2", bufs=2, space="PSUM")
        if True:
            # ---- constants / accumulators ----
            idx_sb = constp.tile([128, IDX_TOT // 16], mybir.dt.int16, tag="idx")
            nc.sync.dma_start(idx_sb[:, :], idx[:, :])
            onesb16 = constp.tile([1, 128], BF16, tag="onesb16")
            nc.vector.memset(onesb16[:, :], 1.0)
            wkbh_sb = constp.tile([1, S, D], BF16, tag="wkbh")
            nc.sync.dma_start(wkbh_sb[:, :, :], wkbH[:, :, :])
            idx2_sb = constp.tile(
                [128, N_PESG * IDX_PER_SG // 16], mybir.dt.int16, tag="idx2"
            )
            nc.sync.dma_start(idx2_sb[:, :], idx2[:, :])
            off2_sb = constp.tile([128, N_PESG * NDOT], F32, tag="off2")
            nc.sync.dma_start(off2_sb[:, :], off2[:, :])
            iota_sb = constp.tile([128, GR], F32, tag="iota")
            nc.sync.dma_start(iota_sb[:, :], iota64[:, :])
            # DRAM scratch for the all-pairs dot matrix [4*128 rows, 3136]
            dots_d = nc.dram_tensor(
                "dotsd", [N_PESG * 128, GR * BLK], F32, kind="Internal"
            )
            dsem = [nc.alloc_semaphore(f"dsem{i}") for i in range(N_PESG)]

            # batched softmax state: one column per supergroup
            mpos_all = accp.tile([128, NSG], F32, tag="mpos")
            negm_all = accp.tile([128, NSG], F32, tag="negm")
            maxneg_all = accp.tile([128, NSG], F32, tag="maxneg")
            ssum_all = accp.tile([128, NSG], F32, tag="ssum")
            pos_all = accp.tile([128, NSG], F32, tag="pos")

            # resident bf16 ctx^T: [128 d_in, 10 d_chunk, NR rows].
            # Split s=0 rows into their own tile so the first matmul group
            # can start as soon as 0.86MB (not 2.9MB) has landed.
            ctx_r = ctxTh[:, :].rearrange("(do di) r -> di do r", di=128)
            ctxh_a = constp.tile([128, 10, NS[0]], BF16, tag="ctxha")
            nc.sync.dma_start(ctxh_a[:, :, :], ctx_r[:, :, 0 : NS[0]])
            ctxh_b = constp.tile([128, 10, NR - NS[0]], BF16, tag="ctxhb")

            pred_tiles = [
                predp.tile([128, D], FP16, tag="pred", name=f"pred{i}")
                for i in range(NSG)
            ]
            # rows 96..127 of the last supergroup are never written by the
            # repack; zero them so phase-2 reads are defined.
            nc.vector.memset(pred_tiles[8][96:128, :], 0.0)

            ench_ap = ench[:, :]
            dstt_tiles = []
            dact_tiles = []
            predT_tiles = []
            pre_gt = {}

            def issue_gather(sg, ci):
                goff, w = GCHUNKS[ci]
                gt = gathp.tile([128, 6, D], FP16, tag="gt")
                pos0 = sg * IDX_PER_SG + goff * 128
                nidx = w * 128
                nc.gpsimd.dma_gather(
                    gt[:, :w, :],
                    ench_ap,
                    idx_sb[:, pos0 // 16 : (pos0 + nidx) // 16],
                    nidx,
                    nidx,
                    D,
                    queue_num=(sg * len(GCHUNKS) + ci) % 2,
                )
                return gt

            # start sg0's gathers before phase 1 so SWDGE streams from t=0
            for ci in range(len(GCHUNKS)):
                pre_gt[(0, ci)] = issue_gather(0, ci)

            def emit_sg(sg):
                """Phase 2 for one 128-row supergroup: dots from gathers."""
                d_stt = dsttp.tile([128, N_STT], F32, tag="dstt", name=f"ds{sg}")
                d_act = dactp.tile(
                    [128, NDOT - N_STT], F32, tag="dact", name=f"da{sg}"
                )
                dstt_tiles.append(d_stt)
                dact_tiles.append(d_act)
                for ci, (goff, w) in enumerate(GCHUNKS):
                    gt = pre_gt.pop((sg, ci), None)
                    if gt is None:
                        gt = issue_gather(sg, ci)
                    for j in range(w):
                        g = goff + j
                        if g < N_STT:
                            scr = scrp.tile([128, D], FP16, tag="scr")
                            # fused dot: out = (gt*1.0)*pred, accum = sum(out)
                            nc.vector.scalar_tensor_tensor(
                                scr[:, :],
                                gt[:, j, :],
                                1.0,
                                pred_tiles[sg][:, :],
                                op0=Alu.mult,
                                op1=Alu.mult,
                                accum_out=d_stt[:, g : g + 1],
                            )
                        else:
                            # product on DVE (fp16 2x), accumulate on ACT
                            scr2 = scr2p.tile([128, D], FP16, tag="scr2")
                            nc.vector.tensor_tensor(
                                scr2[:, :], gt[:, j, :], pred_tiles[sg][:, :],
                                Alu.mult,
                            )
                            dump = scrp.tile([128, D], FP16, tag="dump")
                            nc.scalar.activation(
                                dump[:, :],
                                scr2[:, :],
                                Act.Copy,
                                accum_out=d_act[:, g - N_STT : g - N_STT + 1],
                            )
            # ---- phase 1 (with interleaved phase-2 emission) ----
            E_PARTS = ((0, 512), (512, 512), (1024, 256))
            next_sg = 0
            red_sg = 0

            def emit_reductions(sg):
                """Per-sg small reductions (DVE), run lag-2 behind the dots
                so the DVE FIFO never stalls waiting on ACT accums."""
                d_stt = dstt_tiles[sg]
                d_act = dact_tiles[sg]
                m2 = smallp.tile([128, 1], F32, tag="m2")
                nc.vector.tensor_reduce(m2[:, :], d_act[:, :], Ax.X, Alu.max)
                m3 = smallp.tile([128, 1], F32, tag="m3")
                nc.vector.tensor_reduce(
                    m3[:, :], d_stt[:, 1:N_STT], Ax.X, Alu.max
                )
                nc.vector.tensor_tensor(
                    maxneg_all[:, sg : sg + 1], m3[:, :], m2[:, :], Alu.max
                )
                nc.vector.tensor_tensor(
                    mpos_all[:, sg : sg + 1],
                    maxneg_all[:, sg : sg + 1],
                    d_stt[:, 0:1],
                    Alu.max,
                )
                nc.vector.tensor_copy(pos_all[:, sg : sg + 1], d_stt[:, 0:1])

            for s in range(S):
                wk_r = wkTh[s, :, :].rearrange("(do di) e -> di do e", di=128)
                wk_parts = []
                for eoff, ew in E_PARTS:
                    wkt = wkp.tile([128, 10, ew], BF16, tag=f"wkh{eoff}")
                    nc.sync.dma_start(wkt[:, :, :], wk_r[:, :, eoff : eoff + ew])
                    wk_parts.append(wkt)
                if s == 0:
                    nc.sync.dma_start(
                        ctxh_b[:, :, :], ctx_r[:, :, NS[0] : NR]
                    )
                for t in range(T_S[s]):
                    M = min(128, NS[s] - 128 * t)
                    roff = SOFF[s] + 128 * t
                    if s == 0:
                        ch = ctxh_a[:, :, roff : roff + M]
                    else:
                        ch = ctxh_b[:, :, roff - NS[0] : roff - NS[0] + M]
                    ps = psump.tile([128, D], F32, tag="ps")
                    for d in range(10):
                        for pi, (eoff, ew) in enumerate(E_PARTS):
                            nc.tensor.matmul(
                                ps[:M, eoff : eoff + ew],
                                lhsT=ch[:, d, :],
                                rhs=wk_parts[pi][:, d, :],
                                start=(d == 0), stop=False,
                            )
                    # bias via K=1 matmuls; each PSUM-bank region closes its
                    # accumulation group with stop=True
                    for eoff, ew in E_PARTS:
                        nc.tensor.matmul(
                            ps[:M, eoff : eoff + ew],
                            lhsT=onesb16[0:1, :M],
                            rhs=wkbh_sb[0:1, s, eoff : eoff + ew],
                            start=False, stop=True,
                        )
                    # PSUM -> fp16 staging on the scalar engine
                    stg = stagep.tile([128, D], FP16, tag="stg")
                    nc.scalar.copy(stg[:M, :], ps[:M, :])
                    # repack into dense 128-row supergroup tiles
                    k, p0 = divmod(roff, 128)
                    n1 = min(M, 128 - p0)
                    nc.sync.dma_start(
                        pred_tiles[k][p0 : p0 + n1, :], stg[0:n1, :]
                    )
                    if M > n1:
                        nc.sync.dma_start(
                            pred_tiles[k + 1][0 : M - n1, :], stg[n1:M, :]
                        )
                    covered = roff + M
                    while next_sg < NSG and min(128 * (next_sg + 1), NR) <= covered:
                        if next_sg < PE_SG0:
                            emit_sg(next_sg)
                        else:
                            # predT via HWDGE transpose, interleaved so the
                            # sync queue spreads them across phase 1
                            i = next_sg - PE_SG0
                            pt = predTp.tile(
                                [128, 10, 128], FP16, tag="predT", name=f"pt{i}"
                            )
                            for c in range(10):
                                nc.sync.dma_start_transpose(
                                    pt[:, c, :],
                                    pred_tiles[next_sg][:, c * 128 : (c + 1) * 128],
                                )
                            predT_tiles.append(pt)
                        next_sg += 1
            assert next_sg == NSG

            # ---- all-pairs path for supergroups 5..8 on the PE ----
            # Two halves of 2 sgs: half0's gathers+selects overlap half1's
            # matmuls (encT is streamed twice; DMA has the headroom).
            encT_r = encT[:, :].rearrange("(ec ei) t -> ei ec t", ei=128)
            dots_flat = dots_d[:, :].rearrange("r (b k) -> (r b) k", k=GR)
            dall_tiles = []

            def emit_pe_dots(i):
                da = dallp.tile([128, NDOT], F32, tag="dall", name=f"dl{i}")
                dall_tiles.append(da)
                nc.gpsimd.wait_ge(dsem[i], 7 * 16)
                for ci, (goff, w) in enumerate(GCHUNKS):
                    gt2 = gt2p.tile([128, 6, GR], F32, tag="gt2")
                    pos0 = i * IDX_PER_SG + goff * 128
                    nidx = w * 128
                    nc.gpsimd.dma_gather(
                        gt2[:, :w, :],
                        dots_flat,
                        idx2_sb[:, pos0 // 16 : (pos0 + nidx) // 16],
                        nidx,
                        nidx,
                        GR,
                        queue_num=(i * len(GCHUNKS) + ci) % 2,
                    )
                    for j in range(w):
                        g = goff + j
                        scr3 = scrp.tile([128, GR], F32, tag="scr3")
                        # dot select: (iota == off) * granule, summed
                        nc.vector.scalar_tensor_tensor(
                            scr3[:, :],
                            iota_sb[:, :],
                            off2_sb[:, i * NDOT + g : i * NDOT + g + 1],
                            gt2[:, j, :],
                            op0=Alu.is_equal,
                            op1=Alu.mult,
                            accum_out=da[:, g : g + 1],
                        )

            for h0, h1 in ((0, 2), (2, 4)):
                for tc7 in range(7):
                    t0 = tc7 * TABC
                    encc = enccp.tile([128, 10, TABC], FP16, tag="encc")
                    nc.sync.dma_start(encc[:, :, :], encT_r[:, :, t0 : t0 + TABC])
                    for i in range(h0, h1):
                        ps2 = psum2p.tile([128, TABC], F32, tag="ps2")
                        for e in range(10):
                            nc.tensor.matmul(
                                ps2[:, :],
                                lhsT=predT_tiles[i][:, e, :],
                                rhs=encc[:, e, :],
                                start=(e == 0), stop=(e == 9),
                            )
                        dsb = dsbp.tile([128, TABC], F32, tag="dsb")
                        nc.scalar.copy(dsb[:, :], ps2[:, :])
                        nc.gpsimd.dma_start(
                            dots_d[i * 128 : (i + 1) * 128, t0 : t0 + TABC],
                            dsb[:, :],
                        ).then_inc(dsem[i], 16)
                for i in range(h0, h1):
                    emit_pe_dots(i)
            while red_sg < PE_SG0:
                emit_reductions(red_sg)
                red_sg += 1
            for i in range(N_PESG):
                sg = PE_SG0 + i
                da = dall_tiles[i]
                nc.vector.tensor_reduce(
                    maxneg_all[:, sg : sg + 1], da[:, 1:NDOT], Ax.X, Alu.max
                )
                nc.vector.tensor_tensor(
                    mpos_all[:, sg : sg + 1],
                    maxneg_all[:, sg : sg + 1],
                    da[:, 0:1],
                    Alu.max,
                )
                nc.vector.tensor_copy(pos_all[:, sg : sg + 1], da[:, 0:1])

            # ---- batched softmax-CE tail ----
            nc.vector.tensor_scalar_mul(negm_all[:, :], mpos_all[:, :], -1.0)
            e_t = scrp.tile([128, NDOT], F32, tag="et")
            s2_all = accp.tile([128, NSG], F32, tag="s2")
            nc.vector.memset(s2_all[:, :], 0.0)
            for sg in range(NSG):
                if sg < PE_SG0:
                    nc.scalar.activation(
                        e_t[:, 0:N_STT],
                        dstt_tiles[sg][:, :],
                        Act.Exp,
                        bias=negm_all[:, sg : sg + 1],
                        scale=1.0,
                        accum_out=ssum_all[:, sg : sg + 1],
                    )
                    nc.scalar.activation(
                        e_t[:, N_STT:NDOT],
                        dact_tiles[sg][:, :],
                        Act.Exp,
                        bias=negm_all[:, sg : sg + 1],
                        scale=1.0,
                        accum_out=s2_all[:, sg : sg + 1],
                    )
                else:
                    nc.scalar.activation(
                        e_t[:, :],
                        dall_tiles[sg - PE_SG0][:, :],
                        Act.Exp,
                        bias=negm_all[:, sg : sg + 1],
                        scale=1.0,
                        accum_out=ssum_all[:, sg : sg + 1],
                    )
            stot = accp.tile([128, NSG], F32, tag="stot")
            nc.vector.tensor_tensor(
                stot[:, :], ssum_all[:, :], s2_all[:, :], Alu.add
            )
            lns = accp.tile([128, NSG], F32, tag="lns")
            nc.scalar.activation(lns[:, :], stot[:, :], Act.Ln)
            # loss = ln(sum) + m - pos
            tmp = accp.tile([128, NSG], F32, tag="tmp")
            nc.vector.tensor_tensor(tmp[:, :], lns[:, :], mpos_all[:, :], Alu.add)
            loss_all = accp.tile([128, NSG], F32, tag="loss")
            nc.vector.tensor_tensor(
                loss_all[:, :], tmp[:, :], pos_all[:, :], Alu.subtract
            )
            corr_all = accp.tile([128, NSG], F32, tag="corr")
            nc.vector.tensor_tensor(
                corr_all[:, :], pos_all[:, :], maxneg_all[:, :], Alu.is_ge
            )
            # rows 96..127 of the last supergroup are padding
            nc.vector.memset(loss_all[96:128, NSG - 1 : NSG], 0.0)
            nc.vector.memset(corr_all[96:128, NSG - 1 : NSG], 0.0)
            acc2 = accp.tile([128, 2], F32, tag="acc2")
            nc.vector.tensor_reduce(
                acc2[:, 0:1], loss_all[:, :], Ax.X, Alu.add
            )
            nc.vector.tensor_reduce(
                acc2[:, 1:2], corr_all[:, :], Ax.X, Alu.add
            )

            # per-partition sums go out as [128,2]; host sums partitions
            nc.sync.dma_start(out[:, :], acc2[:, :])

    nc.compile()
    return nc


def _row_targets(core: int, neg_idx: np.ndarray) -> np.ndarray:
    """[NR, 17] int array: flat enc index of positive + 16 negatives per row."""
    tg = np.zeros((NR, NDOT), np.int64)
    ri = 0
    for s in range(S):
        rows = 6 - s
        for b in range(BSH):
            bg = core * BSH + b
            for r in range(rows):
                for c7 in range(G):
                    tg[ri, 0] = bg * G * G + (s + 1 + r) * G + c7
                    tg[ri, 1:] = neg_idx[bg, s, r, c7]
                    ri += 1
    assert ri == NR
    return tg


def _build_idx(core: int, neg_idx: np.ndarray) -> np.ndarray:
    """int16 [128, IDX_TOT//16] gather-index tensor in SWDGE wrap layout."""
    tg = _row_targets(core, neg_idx)
    tg_pad = np.zeros((NSG * 128, NDOT), np.int64)
    tg_pad[:NR] = tg
    # list position sg*2176 + g*128 + p  ->  target of (row sg*128+p, dot g)
    lst = tg_pad.reshape(NSG, 128, NDOT).transpose(0, 2, 1).reshape(-1)
    arr = lst.astype(np.int16).reshape(-1, 16).T  # [16, IDX_TOT//16]
    return np.ascontiguousarray(np.tile(arr, (8, 1)))  # [128, ...]


def _pe_blocks(core: int, neg_idx: np.ndarray):
    """block / offset arrays [N_PESG*128, 17] for the PE-path dots gather."""
    tg = _row_targets(core, neg_idx)
    tg_pad = np.zeros((NSG * 128, NDOT), np.int64)
    tg_pad[:NR] = tg
    tgp = tg_pad[PE_SG0 * 128 :]  # [512, 17]
    lr = np.arange(N_PESG * 128)[:, None]
    blocks = lr * BLK + tgp // GR
    offs = tgp % GR
    return blocks, offs


def _build_idx2(core: int, neg_idx: np.ndarray) -> np.ndarray:
    blocks, _ = _pe_blocks(core, neg_idx)
    # list position i*2176 + g*128 + p -> block of (local row i*128+p, dot g)
    lst = blocks.reshape(N_PESG, 128, NDOT).transpose(0, 2, 1).reshape(-1)
    arr = lst.astype(np.int16).reshape(-1, 16).T
    return np.ascontiguousarray(np.tile(arr, (8, 1)))


def _build_off2(core: int, neg_idx: np.ndarray) -> np.ndarray:
    _, offs = _pe_blocks(core, neg_idx)
    # [128, N_PESG*17]: off2[p, i*17+g]
    o = offs.reshape(N_PESG, 128, NDOT).transpose(1, 0, 2).reshape(128, -1)
    return np.ascontiguousarray(o.astype(np.float32))


def _prep_in_maps(contexts, encodings, Wk_w, Wk_b, neg_idx):
    contexts = np.ascontiguousarray(np.asarray(contexts, np.float32))
    encodings = np.ascontiguousarray(np.asarray(encodings, np.float32))
    Wk_w = np.ascontiguousarray(np.asarray(Wk_w, np.float32))
    Wk_b = np.ascontiguousarray(np.asarray(Wk_b, np.float32))
    neg_idx = np.asarray(neg_idx)

    ench = np.ascontiguousarray(
        encodings.reshape(B * G * G, D).astype(np.float16)
    )
    encT_np = np.ascontiguousarray(ench.T)
    iota_np = np.ascontiguousarray(
        np.tile(np.arange(GR, dtype=np.float32), (128, 1))
    )
    wkT = Wk_w.transpose(0, 2, 1)  # [S, d, e]
    wkTh = np.ascontiguousarray(wkT.astype(ml_dtypes.bfloat16))
    wkbH = np.ascontiguousarray(Wk_b[None, :, :].astype(ml_dtypes.bfloat16))

    in_maps = []
    for c in range(NCORES):
        bs = slice(c * BSH, (c + 1) * BSH)
        ctx_rows = np.concatenate(
            [contexts[bs, : 6 - s].reshape(-1, D) for s in range(S)], axis=0
        )
        ctxT = ctx_rows.T  # [d, NR]
        ctxTh = np.ascontiguousarray(ctxT.astype(ml_dtypes.bfloat16))
        in_maps.append(
            {
                "ctxTh": ctxTh,
                "wkTh": wkTh,
                "wkbH": wkbH,
                "ench": ench,
                "idx": _build_idx(c, neg_idx),
                "encT": encT_np,
                "idx2": _build_idx2(c, neg_idx),
                "off2": _build_off2(c, neg_idx),
                "iota64": iota_np,
            }
        )
    return in_maps


def kernel(contexts, encodings, Wk_w, Wk_b, neg_idx, _trace=False):
    in_maps = _prep_in_maps(contexts, encodings, Wk_w, Wk_b, neg_idx)
    nc = build_nc()
    res = run_bass_kernel_spmd(nc, in_maps, list(range(NCORES)), trace=_trace)
    LAST_RUN["exec_time_ns"] = res.exec_time_ns
    LAST_RUN["results"] = res.results
    loss = np.float32(0.0)
    corr = np.float32(0.0)
    for o in res.results:
        loss += np.float32(o["out"][:, 0].sum(dtype=np.float64))
        corr += np.float32(o["out"][:, 1].sum(dtype=np.float64))
    return (
        np.float32(loss / np.float32(N_PREDS)),
        np.float32(corr / np.float32(N_PREDS)),
    )


# revision 50
# speedup vs baseline: 1.4079x; 1.4079x over previous
"""Trainium2 Bass kernel for the CPC loss problem (nn_CPC_85117661872355).

Strategy (data-parallel over batch B across 8 cores):
  - Each core handles 8 of the 64 batch elements: 1120 prediction rows.
  - pred = ctx @ Wk[s]^T + b on the PE as a single-pass bf16 matmul
    (fp32 PSUM accumulate).  Host-side numpy check against the fixed
    inputs shows 1-pass bf16 + fp16 targets reproduces the reference
    argmax count exactly and has loss rel-err ~3e-6.
  - All 17 logits per row (1 positive + 16 negatives) are dot products
    pred_row . enc_flat[idx].  Target vectors are fetched with SWDGE
    dma_gather from an fp16 copy of the encoding table and the dots are
    split across two engines: 4 slots/supergroup on DVE via the fused
    scalar_tensor_tensor, 13 slots as DVE tensor_tensor (fp16 2x mode)
    followed by an ACT Copy+accumulate readout.  Phase-2 work for each
    128-row supergroup is emitted as soon as phase 1 has produced its
    pred rows, so gather/DVE/ACT/PE all overlap.
  - Softmax-CE runs batched at the end (single Exp table load, one Ln)
    and the argmax==0 check on DVE; per-core (loss_sum, correct_sum)
    are reduced over partitions with a K=128 ones-matmul and DMA'd out
    as [1,2].  Host sums the 8 partial pairs and divides by n_preds.
"""

import functools

import ml_dtypes
import numpy as np

import concourse.bass as bass
import concourse.mybir as mybir
import concourse.tile as tile
from concourse import bacc
from concourse.bass_utils import run_bass_kernel_spmd

F32 = mybir.dt.float32
BF16 = mybir.dt.bfloat16
FP16 = mybir.dt.float16

B, G, D = 64, 7, 1280
S, NEG = 5, 16
NCORES = 8
BSH = B // NCORES  # 8
NS = [BSH * (6 - s) * G for s in range(S)]  # [336, 280, 224, 168, 112]
SOFF = [0]
for n in NS:
    SOFF.append(SOFF[-1] + n)
NR = SOFF[-1]  # 1120 rows per core
NSG = 9  # supergroups of 128 rows
NDOT = 17  # 1 positive + 16 negatives
T_S = [3, 3, 2, 2, 1]  # row-tiles per s
GCHUNKS = [(0, 6), (6, 6), (12, 5)]  # (goff, width)
N_STT = 6  # slots computed via DVE STT; the rest go TT(DVE) + accum(ACT)
IDX_PER_SG = NDOT * 128  # 2176
IDX_TOT = NSG * IDX_PER_SG  # 19584
N_PREDS = B * G * 20  # 8960

# Results of the last device run (for test harness introspection)
LAST_RUN = {}


@functools.lru_cache(maxsize=1)
def build_nc() -> bass.Bass:
    nc = bacc.Bacc(
        "TRN2",
        target_bir_lowering=False,
        debug=False,
        num_devices=NCORES,
        num_swdge_queues=4,
    )
    ctxTh = nc.declare_dram_parameter("ctxTh", [D, NR], BF16, isOutput=False)
    wkTh = nc.declare_dram_parameter("wkTh", [S, D, D], BF16, isOutput=False)
    wkbH = nc.declare_dram_parameter("wkbH", [1, S, D], BF16, isOutput=False)
    ench = nc.declare_dram_parameter("ench", [B * G * G, D], FP16, isOutput=False)
    idx = nc.declare_dram_parameter(
        "idx", [128, IDX_TOT // 16], mybir.dt.int16, isOutput=False
    )
    out = nc.declare_dram_parameter("out", [1, 2], F32, isOutput=True)

    Alu = mybir.AluOpType
    Act = mybir.ActivationFunctionType
    Ax = mybir.AxisListType

    with tile.TileContext(nc) as tc:
        with (
            tc.tile_pool(name="const", bufs=1) as constp,
            tc.tile_pool(name="wk", bufs=2) as wkp,
            tc.tile_pool(name="pred", bufs=NSG) as predp,
            tc.tile_pool(name="stage", bufs=2) as stagep,
            tc.tile_pool(name="gath", bufs=4) as gathp,
            tc.tile_pool(name="scr", bufs=1) as scrp,
            tc.tile_pool(name="scr2", bufs=6) as scr2p,
            tc.tile_pool(name="dstt", bufs=NSG) as dsttp,
            tc.tile_pool(name="dact", bufs=NSG) as dactp,
            tc.tile_pool(name="small", bufs=4) as smallp,
            tc.tile_pool(name="acc", bufs=1) as accp,
            tc.tile_pool(name="psum", bufs=2, space="PSUM") as psump,
            tc.tile_pool(name="psumf", bufs=1, space="PSUM") as psumfp,
        ):
            # ---- constants / accumulators ----
            idx_sb = constp.tile([128, IDX_TOT // 16], mybir.dt.int16, tag="idx")
            nc.sync.dma_start(idx_sb[:, :], idx[:, :])
            ones_sb = constp.tile([128, 1], F32, tag="ones")
            nc.vector.memset(ones_sb[:, :], 1.0)
            onesb16 = constp.tile([1, 128], BF16, tag="onesb16")
            nc.vector.memset(onesb16[:, :], 1.0)
            wkbh_sb = constp.tile([1, S, D], BF16, tag="wkbh")
            nc.sync.dma_start(wkbh_sb[:, :, :], wkbH[:, :, :])

            # batched softmax state: one column per supergroup
            mpos_all = accp.tile([128, NSG], F32, tag="mpos")
            negm_all = accp.tile([128, NSG], F32, tag="negm")
            maxneg_all = accp.tile([128, NSG], F32, tag="maxneg")
            ssum_all = accp.tile([128, NSG], F32, tag="ssum")
            pos_all = accp.tile([128, NSG], F32, tag="pos")

            # resident bf16 ctx^T: [128 d_in, 10 d_chunk, NR rows].
            # Split s=0 rows into their own tile so the first matmul group
            # can start as soon as 0.86MB (not 2.9MB) has landed.
            ctx_r = ctxTh[:, :].rearrange("(do di) r -> di do r", di=128)
            ctxh_a = constp.tile([128, 10, NS[0]], BF16, tag="ctxha")
            nc.sync.dma_start(ctxh_a[:, :, :], ctx_r[:, :, 0 : NS[0]])
            ctxh_b = constp.tile([128, 10, NR - NS[0]], BF16, tag="ctxhb")

            pred_tiles = [
                predp.tile([128, D], FP16, tag="pred", name=f"pred{i}")
                for i in range(NSG)
            ]
            # rows 96..127 of the last supergroup are never written by the
            # repack; zero them so phase-2 reads are defined.
            nc.vector.memset(pred_tiles[8][96:128, :], 0.0)

            ench_ap = ench[:, :]
            dstt_tiles = []
            dact_tiles = []
            pre_gt = {}

            def issue_gather(sg, ci):
                goff, w = GCHUNKS[ci]
                gt = gathp.tile([128, 6, D], FP16, tag="gt")
                pos0 = sg * IDX_PER_SG + goff * 128
                nidx = w * 128
                nc.gpsimd.dma_gather(
                    gt[:, :w, :],
                    ench_ap,
                    idx_sb[:, pos0 // 16 : (pos0 + nidx) // 16],
                    nidx,
                    nidx,
                    D,
                    queue_num=(sg * len(GCHUNKS) + ci) % 2,
                )
                return gt

            # start sg0's gathers before phase 1 so SWDGE streams from t=0
            for ci in range(len(GCHUNKS)):
                pre_gt[(0, ci)] = issue_gather(0, ci)

            def emit_sg(sg):
                """Phase 2 for one 128-row supergroup: dots from gathers."""
                d_stt = dsttp.tile([128, N_STT], F32, tag="dstt", name=f"ds{sg}")
                d_act = dactp.tile(
                    [128, NDOT - N_STT], F32, tag="dact", name=f"da{sg}"
                )
                dstt_tiles.append(d_stt)
                dact_tiles.append(d_act)
                for ci, (goff, w) in enumerate(GCHUNKS):
                    gt = pre_gt.pop((sg, ci), None)
                    if gt is None:
                        gt = issue_gather(sg, ci)
                    for j in range(w):
                        g = goff + j
                        if g < N_STT:
                            scr = scrp.tile([128, D], FP16, tag="scr")
                            # fused dot: out = (gt*1.0)*pred, accum = sum(out)
                            nc.vector.scalar_tensor_tensor(
                                scr[:, :],
                                gt[:, j, :],
                                1.0,
                                pred_tiles[sg][:, :],
                                op0=Alu.mult,
                                op1=Alu.mult,
                                accum_out=d_stt[:, g : g + 1],
                            )
                        else:
                            # product on DVE (fp16 2x), accumulate on ACT
                            scr2 = scr2p.tile([128, D], FP16, tag="scr2")
                            nc.vector.tensor_tensor(
                                scr2[:, :], gt[:, j, :], pred_tiles[sg][:, :],
                                Alu.mult,
                            )
                            dump = scrp.tile([128, D], FP16, tag="dump")
                            nc.scalar.activation(
                                dump[:, :],
                                scr2[:, :],
                                Act.Copy,
                                accum_out=d_act[:, g - N_STT : g - N_STT + 1],
                            )
            # ---- phase 1 (with interleaved phase-2 emission) ----
            E_PARTS = ((0, 512), (512, 512), (1024, 256))
            next_sg = 0
            red_sg = 0

            def emit_reductions(sg):
                """Per-sg small reductions (DVE), run lag-2 behind the dots
                so the DVE FIFO never stalls waiting on ACT accums."""
                d_stt = dstt_tiles[sg]
                d_act = dact_tiles[sg]
                m2 = smallp.tile([128, 1], F32, tag="m2")
                nc.vector.tensor_reduce(m2[:, :], d_act[:, :], Ax.X, Alu.max)
                m3 = smallp.tile([128, 1], F32, tag="m3")
                nc.vector.tensor_reduce(
                    m3[:, :], d_stt[:, 1:N_STT], Ax.X, Alu.max
                )
                nc.vector.tensor_tensor(
                    maxneg_all[:, sg : sg + 1], m3[:, :], m2[:, :], Alu.max
                )
                nc.vector.tensor_tensor(
                    mpos_all[:, sg : sg + 1],
                    maxneg_all[:, sg : sg + 1],
                    d_stt[:, 0:1],
                    Alu.max,
                )
                nc.vector.tensor_copy(pos_all[:, sg : sg + 1], d_stt[:, 0:1])

            for s in range(S):
                wk_r = wkTh[s, :, :].rearrange("(do di) e -> di do e", di=128)
                wk_parts = []
                for eoff, ew in E_PARTS:
                    wkt = wkp.tile([128, 10, ew], BF16, tag=f"wkh{eoff}")
                    nc.sync.dma_start(wkt[:, :, :], wk_r[:, :, eoff : eoff + ew])
                    wk_parts.append(wkt)
                if s == 0:
                    nc.sync.dma_start(
                        ctxh_b[:, :, :], ctx_r[:, :, NS[0] : NR]
                    )
                for t in range(T_S[s]):
                    M = min(128, NS[s] - 128 * t)
                    roff = SOFF[s] + 128 * t
                    if s == 0:
                        ch = ctxh_a[:, :, roff : roff + M]
                    else:
                        ch = ctxh_b[:, :, roff - NS[0] : roff - NS[0] + M]
                    ps = psump.tile([128, D], F32, tag="ps")
                    for d in range(10):
                        for pi, (eoff, ew) in enumerate(E_PARTS):
                            nc.tensor.matmul(
                                ps[:M, eoff : eoff + ew],
                                lhsT=ch[:, d, :],
                                rhs=wk_parts[pi][:, d, :],
                                start=(d == 0), stop=False,
                            )
                    # bias via K=1 matmuls; each PSUM-bank region closes its
                    # accumulation group with stop=True
                    for eoff, ew in E_PARTS:
                        nc.tensor.matmul(
                            ps[:M, eoff : eoff + ew],
                            lhsT=onesb16[0:1, :M],
                            rhs=wkbh_sb[0:1, s, eoff : eoff + ew],
                            start=False, stop=True,
                        )
                    # PSUM -> fp16 staging on the scalar engine
                    stg = stagep.tile([128, D], FP16, tag="stg")
                    nc.scalar.copy(stg[:M, :], ps[:M, :])
                    # repack into dense 128-row supergroup tiles
                    k, p0 = divmod(roff, 128)
                    n1 = min(M, 128 - p0)
                    nc.sync.dma_start(
                        pred_tiles[k][p0 : p0 + n1, :], stg[0:n1, :]
                    )
                    if M > n1:
                        nc.sync.dma_start(
                            pred_tiles[k + 1][0 : M - n1, :], stg[n1:M, :]
                        )
                    covered = roff + M
                    while next_sg < NSG and min(128 * (next_sg + 1), NR) <= covered:
                        emit_sg(next_sg)
                        next_sg += 1
            assert next_sg == NSG
            while red_sg < NSG:
                emit_reductions(red_sg)
                red_sg += 1

            # ---- batched softmax-CE tail ----
            nc.vector.tensor_scalar_mul(negm_all[:, :], mpos_all[:, :], -1.0)
            e_t = scrp.tile([128, NDOT], F32, tag="et")
            for sg in range(NSG):
                nc.scalar.activation(
                    e_t[:, 0:N_STT],
                    dstt_tiles[sg][:, :],
                    Act.Exp,
                    bias=negm_all[:, sg : sg + 1],
                    scale=1.0,
                    accum_out=ssum_all[:, sg : sg + 1],
                )
            s2_all = accp.tile([128, NSG], F32, tag="s2")
            for sg in range(NSG):
                nc.scalar.activation(
                    e_t[:, N_STT:NDOT],
                    dact_tiles[sg][:, :],
                    Act.Exp,
                    bias=negm_all[:, sg : sg + 1],
                    scale=1.0,
                    accum_out=s2_all[:, sg : sg + 1],
                )
            stot = accp.tile([128, NSG], F32, tag="stot")
            nc.vector.tensor_tensor(
                stot[:, :], ssum_all[:, :], s2_all[:, :], Alu.add
            )
            lns = accp.tile([128, NSG], F32, tag="lns")
            nc.scalar.activation(lns[:, :], stot[:, :], Act.Ln)
            # loss = ln(sum) + m - pos
            tmp = accp.tile([128, NSG], F32, tag="tmp")
            nc.vector.tensor_tensor(tmp[:, :], lns[:, :], mpos_all[:, :], Alu.add)
            loss_all = accp.tile([128, NSG], F32, tag="loss")
            nc.vector.tensor_tensor(
                loss_all[:, :], tmp[:, :], pos_all[:, :], Alu.subtract
            )
            corr_all = accp.tile([128, NSG], F32, tag="corr")
            nc.vector.tensor_tensor(
                corr_all[:, :], pos_all[:, :], maxneg_all[:, :], Alu.is_ge
            )
            # rows 96..127 of the last supergroup are padding
            nc.vector.memset(loss_all[96:128, NSG - 1 : NSG], 0.0)
            nc.vector.memset(corr_all[96:128, NSG - 1 : NSG], 0.0)
            acc2 = accp.tile([128, 2], F32, tag="acc2")
            nc.vector.tensor_reduce(
                acc2[:, 0:1], loss_all[:, :], Ax.X, Alu.add
            )
            nc.vector.tensor_reduce(
                acc2[:, 1:2], corr_all[:, :], Ax.X, Alu.add
            )

            # ---- final partition reduce: [128,2] -> [1,2] ----
            psf = psumfp.tile([1, 2], F32, tag="psf")
            nc.tensor.matmul(
                psf[:, :], lhsT=ones_sb[:, 0:1], rhs=acc2[:, :], start=True, stop=True
            )
            outsb = smallp.tile([1, 2], F32, tag="outsb")
            nc.vector.tensor_copy(outsb[:, :], psf[:, :])
            nc.sync.dma_start(out[:, :], outsb[:, :])

    nc.compile()
    return nc


def _row_targets(core: int, neg_idx: np.ndarray) -> np.ndarray:
    """[NR, 17] int array: flat enc index of positive + 16 negatives per row."""
    tg = np.zeros((NR, NDOT), np.int64)
    ri = 0
    for s in range(S):
        rows = 6 - s
        for b in range(BSH):
            bg = core * BSH + b
            for r in range(rows):
                for c7 in range(G):
                    tg[ri, 0] = bg * G * G + (s + 1 + r) * G + c7
                    tg[ri, 1:] = neg_idx[bg, s, r, c7]
                    ri += 1
    assert ri == NR
    return tg


def _build_idx(core: int, neg_idx: np.ndarray) -> np.ndarray:
    """int16 [128, IDX_TOT//16] gather-index tensor in SWDGE wrap layout."""
    tg = _row_targets(core, neg_idx)
    tg_pad = np.zeros((NSG * 128, NDOT), np.int64)
    tg_pad[:NR] = tg
    # list position sg*2176 + g*128 + p  ->  target of (row sg*128+p, dot g)
    lst = tg_pad.reshape(NSG, 128, NDOT).transpose(0, 2, 1).reshape(-1)
    arr = lst.astype(np.int16).reshape(-1, 16).T  # [16, IDX_TOT//16]
    return np.ascontiguousarray(np.tile(arr, (8, 1)))  # [128, ...]


def _prep_in_maps(contexts, encodings, Wk_w, Wk_b, neg_idx):
    contexts = np.ascontiguousarray(np.asarray(contexts, np.float32))
    encodings = np.ascontiguousarray(np.asarray(encodings, np.float32))
    Wk_w = np.ascontiguousarray(np.asarray(Wk_w, np.float32))
    Wk_b = np.ascontiguousarray(np.asarray(Wk_b, np.float32))
    neg_idx = np.asarray(neg_idx)

    ench = np.ascontiguousarray(
        encodings.reshape(B * G * G, D).astype(np.float16)
    )
    wkT = Wk_w.transpose(0, 2, 1)  # [S, d, e]
    wkTh = np.ascontiguousarray(wkT.astype(ml_dtypes.bfloat16))
    wkbH = np.ascontiguousarray(Wk_b[None, :, :].astype(ml_dtypes.bfloat16))

    in_maps = []
    for c in range(NCORES):
        bs = slice(c * BSH, (c + 1) * BSH)
        ctx_rows = np.concatenate(
            [contexts[bs, : 6 - s].reshape(-1, D) for s in range(S)], axis=0
        )
        ctxT = ctx_rows.T  # [d, NR]
        ctxTh = np.ascontiguousarray(ctxT.astype(ml_dtypes.bfloat16))
        in_maps.append(
            {
                "ctxTh": ctxTh,
                "wkTh": wkTh,
                "wkbH": wkbH,
                "ench": ench,
                "idx": _build_idx(c, neg_idx),
            }
        )
    return in_maps


def kernel(contexts, encodings, Wk_w, Wk_b, neg_idx, _trace=False):
    in_maps = _prep_in_maps(contexts, encodings, Wk_w, Wk_b, neg_idx)
    nc = build_nc()
    res = run_bass_kernel_spmd(nc, in_maps, list(range(NCORES)), trace=_trace)
    LAST_RUN["exec_time_ns"] = res.exec_time_ns
    LAST_RUN["results"] = res.results
    loss = np.float32(0.0)
    corr = np.float32(0.0)
    for o in res.results:
        loss += np.float32(o["out"][0, 0])
        corr += np.float32(o["out"][0, 1])
    return (
        np.float32(loss / np.float32(N_PREDS)),
        np.float32(corr / np.float32(N_PREDS)),
    )


# revision 51
# speedup vs baseline: 1.4505x; 1.0302x over previous
"""Trainium2 Bass kernel for the CPC loss problem (nn_CPC_85117661872355).

Strategy (data-parallel over batch B across 8 cores):
  - Each core handles 8 of the 64 batch elements: 1120 prediction rows.
  - pred = ctx @ Wk[s]^T + b on the PE as a single-pass bf16 matmul
    (fp32 PSUM accumulate).  Host-side numpy check against the fixed
    inputs shows 1-pass bf16 + fp16 targets reproduces the reference
    argmax count exactly and has loss rel-err ~3e-6.
  - All 17 logits per row (1 positive + 16 negatives) are dot products
    pred_row . enc_flat[idx].  Target vectors are fetched with SWDGE
    dma_gather from an fp16 copy of the encoding table and the dots are
    split across two engines: 4 slots/supergroup on DVE via the fused
    scalar_tensor_tensor, 13 slots as DVE tensor_tensor (fp16 2x mode)
    followed by an ACT Copy+accumulate readout.  Phase-2 work for each
    128-row supergroup is emitted as soon as phase 1 has produced its
    pred rows, so gather/DVE/ACT/PE all overlap.
  - Softmax-CE runs batched at the end (single Exp table load, one Ln)
    and the argmax==0 check on DVE; per-core (loss_sum, correct_sum)
    are reduced over partitions with a K=128 ones-matmul and DMA'd out
    as [1,2].  Host sums the 8 partial pairs and divides by n_preds.
"""

import functools

import ml_dtypes
import numpy as np

import concourse.bass as bass
import concourse.mybir as mybir
import concourse.tile as tile
from concourse import bacc
from concourse.bass_utils import run_bass_kernel_spmd

F32 = mybir.dt.float32
BF16 = mybir.dt.bfloat16
FP16 = mybir.dt.float16

B, G, D = 64, 7, 1280
S, NEG = 5, 16
NCORES = 8
BSH = B // NCORES  # 8
NS = [BSH * (6 - s) * G for s in range(S)]  # [336, 280, 224, 168, 112]
SOFF = [0]
for n in NS:
    SOFF.append(SOFF[-1] + n)
NR = SOFF[-1]  # 1120 rows per core
NSG = 9  # supergroups of 128 rows
NDOT = 17  # 1 positive + 16 negatives
T_S = [3, 3, 2, 2, 1]  # row-tiles per s
GCHUNKS = [(0, 6), (6, 6), (12, 5)]  # (goff, width)
N_STT = 6  # slots computed via DVE STT; the rest go TT(DVE) + accum(ACT)
IDX_PER_SG = NDOT * 128  # 2176
IDX_TOT = NSG * IDX_PER_SG  # 19584
N_PREDS = B * G * 20  # 8960

# Results of the last device run (for test harness introspection)
LAST_RUN = {}


@functools.lru_cache(maxsize=1)
def build_nc() -> bass.Bass:
    nc = bacc.Bacc(
        "TRN2",
        target_bir_lowering=False,
        debug=False,
        num_devices=NCORES,
        num_swdge_queues=4,
    )
    ctxTh = nc.declare_dram_parameter("ctxTh", [D, NR], BF16, isOutput=False)
    wkTh = nc.declare_dram_parameter("wkTh", [S, D, D], BF16, isOutput=False)
    wkbH = nc.declare_dram_parameter("wkbH", [1, S, D], BF16, isOutput=False)
    ench = nc.declare_dram_parameter("ench", [B * G * G, D], FP16, isOutput=False)
    idx = nc.declare_dram_parameter(
        "idx", [128, IDX_TOT // 16], mybir.dt.int16, isOutput=False
    )
    out = nc.declare_dram_parameter("out", [1, 2], F32, isOutput=True)

    Alu = mybir.AluOpType
    Act = mybir.ActivationFunctionType
    Ax = mybir.AxisListType

    with tile.TileContext(nc) as tc:
        with (
            tc.tile_pool(name="const", bufs=1) as constp,
            tc.tile_pool(name="wk", bufs=2) as wkp,
            tc.tile_pool(name="pred", bufs=NSG) as predp,
            tc.tile_pool(name="stage", bufs=2) as stagep,
            tc.tile_pool(name="gath", bufs=4) as gathp,
            tc.tile_pool(name="scr", bufs=1) as scrp,
            tc.tile_pool(name="scr2", bufs=6) as scr2p,
            tc.tile_pool(name="dstt", bufs=NSG) as dsttp,
            tc.tile_pool(name="dact", bufs=NSG) as dactp,
            tc.tile_pool(name="small", bufs=4) as smallp,
            tc.tile_pool(name="acc", bufs=1) as accp,
            tc.tile_pool(name="psum", bufs=2, space="PSUM") as psump,
            tc.tile_pool(name="psumf", bufs=1, space="PSUM") as psumfp,
        ):
            # ---- constants / accumulators ----
            ones_sb = constp.tile([128, 1], F32, tag="ones")
            nc.vector.memset(ones_sb[:, :], 1.0)
            onesb16 = constp.tile([1, 128], BF16, tag="onesb16")
            nc.vector.memset(onesb16[:, :], 1.0)
            wkbh_sb = constp.tile([1, S, D], BF16, tag="wkbh")
            nc.sync.dma_start(wkbh_sb[:, :, :], wkbH[:, :, :])

            # batched softmax state: one column per supergroup
            mpos_all = accp.tile([128, NSG], F32, tag="mpos")
            negm_all = accp.tile([128, NSG], F32, tag="negm")
            maxneg_all = accp.tile([128, NSG], F32, tag="maxneg")
            ssum_all = accp.tile([128, NSG], F32, tag="ssum")
            pos_all = accp.tile([128, NSG], F32, tag="pos")

            # resident bf16 ctx^T: [128 d_in, 10 d_chunk, NR rows].
            # Split s=0 rows into their own tile so the first matmul group
            # can start as soon as 0.86MB (not 2.9MB) has landed.
            ctx_r = ctxTh[:, :].rearrange("(do di) r -> di do r", di=128)
            ctxh_a = constp.tile([128, 10, NS[0]], BF16, tag="ctxha")
            nc.sync.dma_start(ctxh_a[:, :, :], ctx_r[:, :, 0 : NS[0]])
            ctxh_b = constp.tile([128, 10, NR - NS[0]], BF16, tag="ctxhb")
            # s=0 weights ahead of the idx load on the sync FIFO: gathers
            # data-depend on idx, so they cannot outrace these transfers
            wk0_r = wkTh[0, :, :].rearrange("(do di) e -> di do e", di=128)
            wk0_parts = []
            for eoff, ew in ((0, 512), (512, 512), (1024, 256)):
                wkt = wkp.tile([128, 10, ew], BF16, tag=f"wkh{eoff}")
                nc.sync.dma_start(wkt[:, :, :], wk0_r[:, :, eoff : eoff + ew])
                wk0_parts.append(wkt)
            idx_sb = constp.tile([128, IDX_TOT // 16], mybir.dt.int16, tag="idx")
            nc.sync.dma_start(idx_sb[:, :], idx[:, :])

            pred_tiles = [
                predp.tile([128, D], FP16, tag="pred", name=f"pred{i}")
                for i in range(NSG)
            ]
            # rows 96..127 of the last supergroup are never written by the
            # repack; zero them so phase-2 reads are defined.
            nc.vector.memset(pred_tiles[8][96:128, :], 0.0)

            ench_ap = ench[:, :]
            dstt_tiles = []
            dact_tiles = []
            pre_gt = {}

            def issue_gather(sg, ci):
                goff, w = GCHUNKS[ci]
                gt = gathp.tile([128, 6, D], FP16, tag="gt")
                pos0 = sg * IDX_PER_SG + goff * 128
                nidx = w * 128
                nc.gpsimd.dma_gather(
                    gt[:, :w, :],
                    ench_ap,
                    idx_sb[:, pos0 // 16 : (pos0 + nidx) // 16],
                    nidx,
                    nidx,
                    D,
                    queue_num=(sg * len(GCHUNKS) + ci) % 2,
                )
                return gt

            # start sg0's gathers before phase 1 so SWDGE streams from t=0
            for ci in range(len(GCHUNKS)):
                pre_gt[(0, ci)] = issue_gather(0, ci)

            def emit_sg(sg):
                """Phase 2 for one 128-row supergroup: dots from gathers."""
                d_stt = dsttp.tile([128, N_STT], F32, tag="dstt", name=f"ds{sg}")
                d_act = dactp.tile(
                    [128, NDOT - N_STT], F32, tag="dact", name=f"da{sg}"
                )
                dstt_tiles.append(d_stt)
                dact_tiles.append(d_act)
                for ci, (goff, w) in enumerate(GCHUNKS):
                    gt = pre_gt.pop((sg, ci), None)
                    if gt is None:
                        gt = issue_gather(sg, ci)
                    for j in range(w):
                        g = goff + j
                        if g < N_STT:
                            scr = scrp.tile([128, D], FP16, tag="scr")
                            # fused dot: out = (gt*1.0)*pred, accum = sum(out)
                            nc.vector.scalar_tensor_tensor(
                                scr[:, :],
                                gt[:, j, :],
                                1.0,
                                pred_tiles[sg][:, :],
                                op0=Alu.mult,
                                op1=Alu.mult,
                                accum_out=d_stt[:, g : g + 1],
                            )
                        else:
                            # product on DVE (fp16 2x), accumulate on ACT
                            scr2 = scr2p.tile([128, D], FP16, tag="scr2")
                            nc.vector.tensor_tensor(
                                scr2[:, :], gt[:, j, :], pred_tiles[sg][:, :],
                                Alu.mult,
                            )
                            dump = scrp.tile([128, D], FP16, tag="dump")
                            nc.scalar.activation(
                                dump[:, :],
                                scr2[:, :],
                                Act.Copy,
                                accum_out=d_act[:, g - N_STT : g - N_STT + 1],
                            )
            # ---- phase 1 (with interleaved phase-2 emission) ----
            E_PARTS = ((0, 512), (512, 512), (1024, 256))
            next_sg = 0
            red_sg = 0

            def emit_reductions(sg):
                """Per-sg small reductions (DVE), run lag-2 behind the dots
                so the DVE FIFO never stalls waiting on ACT accums."""
                d_stt = dstt_tiles[sg]
                d_act = dact_tiles[sg]
                m2 = smallp.tile([128, 1], F32, tag="m2")
                nc.vector.tensor_reduce(m2[:, :], d_act[:, :], Ax.X, Alu.max)
                m3 = smallp.tile([128, 1], F32, tag="m3")
                nc.vector.tensor_reduce(
                    m3[:, :], d_stt[:, 1:N_STT], Ax.X, Alu.max
                )
                nc.vector.tensor_tensor(
                    maxneg_all[:, sg : sg + 1], m3[:, :], m2[:, :], Alu.max
                )
                nc.vector.tensor_tensor(
                    mpos_all[:, sg : sg + 1],
                    maxneg_all[:, sg : sg + 1],
                    d_stt[:, 0:1],
                    Alu.max,
                )
                nc.vector.tensor_copy(pos_all[:, sg : sg + 1], d_stt[:, 0:1])

            for s in range(S):
                if s == 0:
                    wk_parts = wk0_parts
                    nc.sync.dma_start(
                        ctxh_b[:, :, :], ctx_r[:, :, NS[0] : NR]
                    )
                else:
                    wk_r = wkTh[s, :, :].rearrange("(do di) e -> di do e", di=128)
                    wk_parts = []
                    for eoff, ew in E_PARTS:
                        wkt = wkp.tile([128, 10, ew], BF16, tag=f"wkh{eoff}")
                        nc.sync.dma_start(
                            wkt[:, :, :], wk_r[:, :, eoff : eoff + ew]
                        )
                        wk_parts.append(wkt)
                for t in range(T_S[s]):
                    M = min(128, NS[s] - 128 * t)
                    roff = SOFF[s] + 128 * t
                    if s == 0:
                        ch = ctxh_a[:, :, roff : roff + M]
                    else:
                        ch = ctxh_b[:, :, roff - NS[0] : roff - NS[0] + M]
                    ps = psump.tile([128, D], F32, tag="ps")
                    for d in range(10):
                        for pi, (eoff, ew) in enumerate(E_PARTS):
                            nc.tensor.matmul(
                                ps[:M, eoff : eoff + ew],
                                lhsT=ch[:, d, :],
                                rhs=wk_parts[pi][:, d, :],
                                start=(d == 0), stop=False,
                            )
                    # bias via K=1 matmuls; each PSUM-bank region closes its
                    # accumulation group with stop=True
                    for eoff, ew in E_PARTS:
                        nc.tensor.matmul(
                            ps[:M, eoff : eoff + ew],
                            lhsT=onesb16[0:1, :M],
                            rhs=wkbh_sb[0:1, s, eoff : eoff + ew],
                            start=False, stop=True,
                        )
                    # PSUM -> fp16 staging on the scalar engine
                    stg = stagep.tile([128, D], FP16, tag="stg")
                    nc.scalar.copy(stg[:M, :], ps[:M, :])
                    # repack into dense 128-row supergroup tiles
                    k, p0 = divmod(roff, 128)
                    n1 = min(M, 128 - p0)
                    nc.sync.dma_start(
                        pred_tiles[k][p0 : p0 + n1, :], stg[0:n1, :]
                    )
                    if M > n1:
                        nc.sync.dma_start(
                            pred_tiles[k + 1][0 : M - n1, :], stg[n1:M, :]
                        )
                    covered = roff + M
                    while next_sg < NSG and min(128 * (next_sg + 1), NR) <= covered:
                        emit_sg(next_sg)
                        next_sg += 1
            assert next_sg == NSG
            while red_sg < NSG - 2:
                emit_reductions(red_sg)
                red_sg += 1
            while red_sg < NSG:
                emit_reductions(red_sg)
                red_sg += 1

            # ---- batched softmax-CE tail ----
            nc.vector.tensor_scalar_mul(negm_all[:, :], mpos_all[:, :], -1.0)
            e_t = scrp.tile([128, NDOT], F32, tag="et")
            for sg in range(NSG):
                nc.scalar.activation(
                    e_t[:, 0:N_STT],
                    dstt_tiles[sg][:, :],
                    Act.Exp,
                    bias=negm_all[:, sg : sg + 1],
                    scale=1.0,
                    accum_out=ssum_all[:, sg : sg + 1],
                )
            s2_all = accp.tile([128, NSG], F32, tag="s2")
            for sg in range(NSG):
                nc.scalar.activation(
                    e_t[:, N_STT:NDOT],
                    dact_tiles[sg][:, :],
                    Act.Exp,
                    bias=negm_all[:, sg : sg + 1],
                    scale=1.0,
                    accum_out=s2_all[:, sg : sg + 1],
                )
            stot = accp.tile([128, NSG], F32, tag="stot")
            nc.vector.tensor_tensor(
                stot[:, :], ssum_all[:, :], s2_all[:, :], Alu.add
            )
            lns = accp.tile([128, NSG], F32, tag="lns")
            nc.scalar.activation(lns[:, :], stot[:, :], Act.Ln)
            # loss = ln(sum) + m - pos
            tmp = accp.tile([128, NSG], F32, tag="tmp")
            nc.vector.tensor_tensor(tmp[:, :], lns[:, :], mpos_all[:, :], Alu.add)
            loss_all = accp.tile([128, NSG], F32, tag="loss")
            nc.vector.tensor_tensor(
                loss_all[:, :], tmp[:, :], pos_all[:, :], Alu.subtract
            )
            corr_all = accp.tile([128, NSG], F32, tag="corr")
            nc.vector.tensor_tensor(
                corr_all[:, :], pos_all[:, :], maxneg_all[:, :], Alu.is_ge
            )
            # rows 96..127 of the last supergroup are padding
            nc.vector.memset(loss_all[96:128, NSG - 1 : NSG], 0.0)
            nc.vector.memset(corr_all[96:128, NSG - 1 : NSG], 0.0)
            acc2 = accp.tile([128, 2], F32, tag="acc2")
            nc.vector.tensor_reduce(
                acc2[:, 0:1], loss_all[:, :], Ax.X, Alu.add
            )
            nc.vector.tensor_reduce(
                acc2[:, 1:2], corr_all[:, :], Ax.X, Alu.add
            )

            # ---- final partition reduce: [128,2] -> [1,2] ----
            psf = psumfp.tile([1, 2], F32, tag="psf")
            nc.tensor.matmul(
                psf[:, :], lhsT=ones_sb[:, 0:1], rhs=acc2[:, :], start=True, stop=True
            )
            outsb = smallp.tile([1, 2], F32, tag="outsb")
            nc.vector.tensor_copy(outsb[:, :], psf[:, :])
            nc.sync.dma_start(out[:, :], outsb[:, :])

    nc.compile()
    return nc


def _row_targets(core: int, neg_idx: np.ndarray) -> np.ndarray:
    """[NR, 17] int array: flat enc index of positive + 16 negatives per row."""
    tg = np.zeros((NR, NDOT), np.int64)
    ri = 0
    for s in range(S):
        rows = 6 - s
        for b in range(BSH):
            bg = core * BSH + b
            for r in range(rows):
                for c7 in range(G):
                    tg[ri, 0] = bg * G * G + (s + 1 + r) * G + c7
                    tg[ri, 1:] = neg_idx[bg, s, r, c7]
                    ri += 1
    assert ri == NR
    return tg


def _build_idx(core: int, neg_idx: np.ndarray) -> np.ndarray:
    """int16 [128, IDX_TOT//16] gather-index tensor in SWDGE wrap layout."""
    tg = _row_targets(core, neg_idx)
    tg_pad = np.zeros((NSG * 128, NDOT), np.int64)
    tg_pad[:NR] = tg
    # list position sg*2176 + g*128 + p  ->  target of (row sg*128+p, dot g)
    lst = tg_pad.reshape(NSG, 128, NDOT).transpose(0, 2, 1).reshape(-1)
    arr = lst.astype(np.int16).reshape(-1, 16).T  # [16, IDX_TOT//16]
    return np.ascontiguousarray(np.tile(arr, (8, 1)))  # [128, ...]


def _prep_in_maps(contexts, encodings, Wk_w, Wk_b, neg_idx):
    contexts = np.ascontiguousarray(np.asarray(contexts, np.float32))
    encodings = np.ascontiguousarray(np.asarray(encodings, np.float32))
    Wk_w = np.ascontiguousarray(np.asarray(Wk_w, np.float32))
    Wk_b = np.ascontiguousarray(np.asarray(Wk_b, np.float32))
    neg_idx = np.asarray(neg_idx)

    ench = np.ascontiguousarray(
        encodings.reshape(B * G * G, D).astype(np.float16)
    )
    wkT = Wk_w.transpose(0, 2, 1)  # [S, d, e]
    wkTh = np.ascontiguousarray(wkT.astype(ml_dtypes.bfloat16))
    wkbH = np.ascontiguousarray(Wk_b[None, :, :].astype(ml_dtypes.bfloat16))

    in_maps = []
    for c in range(NCORES):
        bs = slice(c * BSH, (c + 1) * BSH)
        ctx_rows = np.concatenate(
            [contexts[bs, : 6 - s].reshape(-1, D) for s in range(S)], axis=0
        )
        ctxT = ctx_rows.T  # [d, NR]
        ctxTh = np.ascontiguousarray(ctxT.astype(ml_dtypes.bfloat16))
        in_maps.append(
            {
                "ctxTh": ctxTh,
                "wkTh": wkTh,
                "wkbH": wkbH,
                "ench": ench,
                "idx": _build_idx(c, neg_idx),
            }
        )
    return in_maps


def kernel(contexts, encodings, Wk_w, Wk_b, neg_idx, _trace=False):
    in_maps = _prep_in_maps(contexts, encodings, Wk_w, Wk_b, neg_idx)
    nc = build_nc()
    res = run_bass_kernel_spmd(nc, in_maps, list(range(NCORES)), trace=_trace)
    LAST_RUN["exec_time_ns"] = res.exec_time_ns
    LAST_RUN["results"] = res.results
    loss = np.float32(0.0)
    corr = np.float32(0.0)
    for o in res.results:
        loss += np.float32(o["out"][0, 0])
        corr += np.float32(o["out"][0, 1])
    return (
        np.float32(loss / np.float32(N_PREDS)),
        np.float32(corr / np.float32(N_PREDS)),
    )
